# revision 1
# baseline (speedup 1.0000x reference)
"""Chamfer distance L2 (B=4, N=M=8192, D=3) on 8 TRN2 NeuronCores.

Sharding: core c handles batch b = c//2, xyz1-half h = c%2 (4096 query
points against all 8192 xyz2 points of the same batch).

Device kernel (per core, identical SPMD program):
  d[n,m] = ||x1[n]||^2 + ||x2[m]||^2 - 2<x1[n],x2[m]> via ONE K=18
  augmented bf16 matmul per output tile. Each coordinate is split into
  bf16 hi+lo (x ~= xh+xl to ~2^-18 rel) and each squared-norm row into
  three bf16 terms (~2^-27 rel), so every partial product is exact in
  the f32 PSUM accumulation - near-f32 accuracy at full bf16 PE rate
  (output-bound: 1 cycle/row regardless of K<=128).
    k 0..2 :  1,1,1        x  s2_h,s2_m,s2_l
    k 3..5 :  s1_h,s1_m,s1_l x  1,1,1
    k 6..8 :  -2*x_h       x  y_h
    k 9..11: -2*x_h        x  y_l
    k12..14: -2*x_l        x  y_h
    k15..17: -2*x_l        x  y_l
  - PE writes d tiles [128 x 2048] into PSUM (4 x N=512 matmuls).
  - ScalarE copies each PSUM chunk to SBUF bf16 (relative rounding of
    candidate distances only - harmless to min selection and value).
  - VectorE: row-min stream (dist1) via bf16 tensor_tensor(min) at 2x
    + per-n-tile finalize, and the column-min accumulator (dist2).
  - Tail: PE transposes colacc 128x128 blocks to PSUM; VectorE strided-
    reduces old-partition axis -> dist2 partials.
Host: means + min-combine of the two per-batch halves (O(N) work only).
"""

import sys

for _p in ("/opt/trn_rl_repo",):
    if _p not in sys.path:
        sys.path.insert(0, _p)

from contextlib import ExitStack

import numpy as np
import ml_dtypes

import concourse.bacc as bacc
import concourse.bass as bass
import concourse.mybir as mybir
import concourse.tile as tile
from concourse import masks
from concourse.bass_utils import run_bass_kernel_spmd

WEIGHT = 0.6
B = 4
N = 8192  # xyz1 points per batch
M = 8192  # xyz2 points per batch
D = 3
NCORES = 8
HALF = N // 2  # xyz1 rows per core = 4096

P = 128  # partitions
NT = HALF // P  # 32 n-tiles per core
CHUNK = 2048  # psum chunk free size (4 banks)
MC = M // CHUNK  # 4 m-chunks
MM_FREE = 512  # one PSUM bank of f32
K = 18  # augmented contraction dim (split-bf16)
GPS_M0 = M  # colacc m >= GPS_M0 merged via gpsimd SWDGE dma-accum; below: Vector

F32 = mybir.dt.float32
BF16 = mybir.dt.bfloat16
MIN = mybir.AluOpType.min
AX = mybir.AxisListType.X
BF = ml_dtypes.bfloat16

_cached = None


def _build():
    """Build + compile the single-core Bass program (shared by all 8 cores)."""
    nc = bacc.Bacc(
        "TRN2",
        target_bir_lowering=False,
        debug=False,
        enable_asserts=False,
        num_devices=NCORES,
    )

    lhs_d = nc.dram_tensor("lhs", [K, HALF], BF16, kind="ExternalInput")
    rhs_d = nc.dram_tensor("rhs", [K, M], BF16, kind="ExternalInput")
    out1_d = nc.dram_tensor("out1", [P, NT], F32, kind="ExternalOutput")
    out2_d = nc.dram_tensor("out2", [P, M // P], F32, kind="ExternalOutput")

    with tile.TileContext(nc) as tc, ExitStack() as ctx:
        const = ctx.enter_context(tc.tile_pool(name="const", bufs=1))
        ckpool = ctx.enter_context(tc.tile_pool(name="ck", bufs=12))
        rapool = ctx.enter_context(tc.tile_pool(name="ra", bufs=3))
        psum = ctx.enter_context(tc.tile_pool(name="ps", bufs=2, space="PSUM"))

        lhs_sb = const.tile([K, HALF], BF16)
        rhs_sb = const.tile([K, M], BF16)
        ident = const.tile([P, P], BF16)
        colacc = const.tile([P, M], BF16)
        dist1 = const.tile([P, NT], F32)
        dist2 = const.tile([P, M // P], F32)

        nc.sync.dma_start(lhs_sb[:], lhs_d[:])
        nc.sync.dma_start(rhs_sb[:], rhs_d[:])
        masks.make_identity(nc, ident[:])

        for nt in range(NT):
            lhsT = lhs_sb[:, nt * P : (nt + 1) * P]
            ra = rapool.tile([P, 1024], BF16, tag="ra")
            rb = rapool.tile([P, 512], BF16, tag="rb")
            for mc in range(MC):
                pt = psum.tile([P, CHUNK], F32, tag="ps")
                for j in range(CHUNK // MM_FREE):
                    m0 = mc * CHUNK + j * MM_FREE
                    nc.tensor.matmul(
                        pt[:, j * MM_FREE : (j + 1) * MM_FREE],
                        lhsT,
                        rhs_sb[:, m0 : m0 + MM_FREE],
                        start=True,
                        stop=True,
                    )
                # PSUM f32 -> SBUF bf16 (ScalarE). First n-tile seeds colacc.
                if nt == 0:
                    dst = colacc[:, mc * CHUNK : (mc + 1) * CHUNK]
                else:
                    ck = ckpool.tile([P, CHUNK], BF16, tag="ck")
                    dst = ck[:]
                nc.scalar.copy(dst, pt[:])
                # row-min stream (dist1) on VectorE, bf16 2x rate
                if mc == 0:
                    nc.vector.tensor_tensor(
                        ra[:], dst[:, 0:1024], dst[:, 1024:2048], MIN
                    )
                else:
                    nc.vector.tensor_tensor(ra[:], ra[:], dst[:, 0:1024], MIN)
                    nc.vector.tensor_tensor(ra[:], ra[:], dst[:, 1024:2048], MIN)
                # column-min accumulator on VectorE
                if nt > 0:
                    ca = colacc[:, mc * CHUNK : (mc + 1) * CHUNK]
                    nc.vector.tensor_tensor(ca, ca, dst, MIN)
            nc.vector.tensor_tensor(rb[:], ra[:, 0:512], ra[:, 512:1024], MIN)
            nc.vector.tensor_reduce(dist1[:, nt : nt + 1], rb[:], axis=AX, op=MIN)

        # dist2 tail: transpose colacc 128x128 blocks, reduce old partitions
        for g in range(M // P // 8):
            tp = psum.tile([P, 8 * P], BF16, tag="ps")
            for b in range(8):
                t = g * 8 + b
                nc.tensor.transpose(
                    tp[:, b * P : (b + 1) * P],
                    colacc[:, t * P : (t + 1) * P],
                    ident[:],
                )
            nc.vector.tensor_reduce(
                dist2[:, g * 8 : (g + 1) * 8],
                tp[:].rearrange("p (b x) -> p b x", x=P),
                axis=AX,
                op=MIN,
            )

        nc.sync.dma_start(out1_d[:], dist1[:])
        nc.sync.dma_start(out2_d[:], dist2[:])

    nc.compile()
    return nc


def _get_nc():
    global _cached
    if _cached is None:
        _cached = _build()
    return _cached


def _split3(v):
    """Split f64 vector into three bf16 terms summing to v to ~2^-27 rel."""
    h = v.astype(BF)
    r = v - h.astype(np.float64)
    m = r.astype(BF)
    l = (r - m.astype(np.float64)).astype(BF)
    return h, m, l


def _in_maps(xyz1, xyz2):
    xyz1 = np.ascontiguousarray(np.asarray(xyz1, dtype=np.float32))
    xyz2 = np.ascontiguousarray(np.asarray(xyz2, dtype=np.float32))
    maps = []
    for c in range(NCORES):
        b, h = divmod(c, 2)
        X = xyz1[b, h * HALF : (h + 1) * HALF].astype(np.float64)  # [4096, 3]
        Y = xyz2[b].astype(np.float64)  # [8192, 3]

        xh = X.astype(BF)
        xl = (X - xh.astype(np.float64)).astype(BF)
        yh = Y.astype(BF)
        yl = (Y - yh.astype(np.float64)).astype(BF)
        Xr = xh.astype(np.float64) + xl.astype(np.float64)  # representable x
        Yr = yh.astype(np.float64) + yl.astype(np.float64)
        s1h, s1m, s1l = _split3(np.einsum("nd,nd->n", Xr, Xr))
        s2h, s2m, s2l = _split3(np.einsum("md,md->m", Yr, Yr))

        lhs = np.empty((K, HALF), BF)
        lhs[0:3] = 1.0
        lhs[3] = s1h
        lhs[4] = s1m
        lhs[5] = s1l
        lhs[6:9] = (-2.0 * xh.astype(np.float64)).astype(BF).T  # exact *2
        lhs[9:12] = lhs[6:9]
        lhs[12:15] = (-2.0 * xl.astype(np.float64)).astype(BF).T
        lhs[15:18] = lhs[12:15]

        rhs = np.empty((K, M), BF)
        rhs[0] = s2h
        rhs[1] = s2m
        rhs[2] = s2l
        rhs[3:6] = 1.0
        rhs[6:9] = yh.T
        rhs[9:12] = yl.T
        rhs[12:15] = yh.T
        rhs[15:18] = yl.T
        maps.append({"lhs": lhs, "rhs": rhs})
    return maps


def _combine(results):
    # dist1: all 8 cores' values are final; out1[p, t] = dist1[t*128 + p]
    d1 = np.concatenate([results[c]["out1"].T.reshape(-1) for c in range(NCORES)])
    # dist2: min over the two half-cores of each batch
    d2 = np.concatenate(
        [
            np.minimum(results[2 * b]["out2"], results[2 * b + 1]["out2"]).T.reshape(-1)
            for b in range(B)
        ]
    )
    val = WEIGHT * (np.float64(d1.mean()) + np.float64(d2.mean())) / 2.0
    return np.float32(val)


def run(xyz1, xyz2, trace=False, **spmd_kwargs):
    """Run on hardware; returns (output_scalar, BassKernelResults)."""
    nc = _get_nc()
    br = run_bass_kernel_spmd(
        nc, _in_maps(xyz1, xyz2), list(range(NCORES)), trace=trace, **spmd_kwargs
    )
    return _combine(br.results), br


def kernel(xyz1, xyz2):
    out, _ = run(xyz1, xyz2)
    return out


if __name__ == "__main__":
    rng = np.random.default_rng(0)
    a = rng.standard_normal((B, N, D)).astype(np.float32)
    b = rng.standard_normal((B, M, D)).astype(np.float32)
    print(kernel(a, b))



# revision 4
# speedup vs baseline: 1.0001x; 1.0001x over previous
"""Chamfer distance L2 (B=4, N=M=8192, D=3) on 8 TRN2 NeuronCores — v2.

Sharding: core c handles batch b = c//2, xyz1-half h = c%2 (4096 query
points against all 8192 xyz2 points of the same batch).

Device kernel (per core, identical SPMD program):
  d[n,m] = ||x1[n]||^2 + ||x2[m]||^2 - 2<x1[n],x2[m]> via ONE K=18
  augmented bf16 matmul per 512-col tile (split-bf16 hi/lo encoding of
  coords and squared norms -> near-f32 accuracy at bf16 PE rate).
  - PE row-tiling: lhs/rhs replicated at partition strips 0/32/64/96;
    the 4 sub-matmuls of each [128 x 2048] PSUM chunk run CONCURRENTLY
    in four 32-row PE strips (tile_position) -> PE busy 248us -> 88us.
  - ScalarE drains PSUM f32 -> SBUF bf16, one [128 x 8192] ck tile per
    n-tile (~212us; at the 1 elem/cycle ACTIVATE floor).
  - VectorE (the binding engine, ~308us busy at 2x_1P tensor_tensor):
    row-min stream per n-tile at FD=2048 + fold + reduce -> dist1[:,nt];
    column-min as level-1 merges of adjacent n-tile pairs (FD=4096)
    plus a pair-lagged accumulate into colacc (keeps V fed while pair
    merges land).
  - Tail: PE transposes colacc 128x128 blocks to PSUM bf16; VectorE
    strided-reduces the old-partition axis -> dist2 partials, with the
    final pair merge interleaved group-by-group for overlap.
  Measured (8-core SPMD): HW exec ~332us, rel err 2.2e-4.

  Rejected offload paths (all probed on this toolchain/HW): SDMA CCE
  accum supports ADD only (min/max fail walrus birverifier); Pool-engine
  TENSOR_TENSOR fails walrus codegen (no Q7 TT opcode); DVE
  tensor_tensor_reduce wedges the device (NRT_EXEC_UNIT_UNRECOVERABLE);
  DVE pool/max8 run at 1x (slower than the 2x tensor_tensor stream).
Host: means + min-combine of the two per-batch halves (O(N) work only).
"""

import sys

for _p in ("/opt/trn_rl_repo",):
    if _p not in sys.path:
        sys.path.insert(0, _p)

from contextlib import ExitStack

import numpy as np
import ml_dtypes

import concourse.bacc as bacc
import concourse.bass as bass
import concourse.mybir as mybir
import concourse.tile as tile
from concourse import masks
from concourse.bass_utils import run_bass_kernel_spmd

WEIGHT = 0.6
B = 4
N = 8192  # xyz1 points per batch
M = 8192  # xyz2 points per batch
D = 3
NCORES = 8
HALF = N // 2  # xyz1 rows per core = 4096

P = 128  # partitions
NT = HALF // P  # 32 n-tiles per core
CHUNK = 2048  # psum chunk free size (4 banks)
MC = M // CHUNK  # 4 m-chunks
MM_FREE = 512  # one PSUM bank of f32
K = 18  # augmented contraction dim (split-bf16)

F32 = mybir.dt.float32
BF16 = mybir.dt.bfloat16
MIN = mybir.AluOpType.min
AX = mybir.AxisListType.X
BF = ml_dtypes.bfloat16

_cached = None


def _build():
    """Build + compile the single-core Bass program (shared by all 8 cores)."""
    nc = bacc.Bacc(
        "TRN2",
        target_bir_lowering=False,
        debug=False,
        enable_asserts=False,
        num_devices=NCORES,
    )

    lhs_d = nc.dram_tensor("lhs", [P, HALF], BF16, kind="ExternalInput")
    rhs_d = nc.dram_tensor("rhs", [P, M], BF16, kind="ExternalInput")
    out1_d = nc.dram_tensor("out1", [P, NT], F32, kind="ExternalOutput")
    out2_d = nc.dram_tensor("out2", [P, M // P], F32, kind="ExternalOutput")

    with tile.TileContext(nc) as tc, ExitStack() as ctx:
        const = ctx.enter_context(tc.tile_pool(name="const", bufs=1))
        ckpool = ctx.enter_context(tc.tile_pool(name="ck", bufs=5))
        rapool = ctx.enter_context(tc.tile_pool(name="ra", bufs=3))
        pmpool = ctx.enter_context(tc.tile_pool(name="pm", bufs=2))
        psum = ctx.enter_context(tc.tile_pool(name="ps", bufs=2, space="PSUM"))

        lhs_sb = const.tile([P, HALF], BF16)
        rhs_sb = const.tile([P, M], BF16)
        ident = const.tile([P, P], BF16)
        colacc = const.tile([P, M], BF16)
        dist1 = const.tile([P, NT], F32)
        dist2 = const.tile([P, M // P], F32)

        # first n-tile's weights + first rhs chunk land first -> early start
        nc.sync.dma_start(lhs_sb[:, 0:P], lhs_d[:, 0:P])
        nc.sync.dma_start(rhs_sb[:, 0:CHUNK], rhs_d[:, 0:CHUNK])
        nc.sync.dma_start(lhs_sb[:, P:HALF], lhs_d[:, P:HALF])
        for mc in range(1, MC):
            nc.sync.dma_start(
                rhs_sb[:, mc * CHUNK : (mc + 1) * CHUNK],
                rhs_d[:, mc * CHUNK : (mc + 1) * CHUNK],
            )
        masks.make_identity(nc, ident[:])

        # pending column ops, emitted with one-pair lag so VectorE keeps
        # row-min work in front of it while L1 pair merges land.
        pending = []

        def flush_pending():
            for kind, src_ap in pending:
                for g in range(2):
                    ca = colacc[:, g * 4096 : (g + 1) * 4096]
                    src = src_ap[:, g * 4096 : (g + 1) * 4096]
                    if kind == "seed":
                        nc.vector.tensor_copy(ca, src)
                    else:
                        nc.vector.tensor_tensor(ca, ca, src, MIN)
            pending.clear()

        prev_ck = None  # ck handle of the even n-tile of the current pair
        for nt in range(NT):
            i, half = divmod(nt, 2)
            ck = ckpool.tile([P, M], BF16, tag="ck")
            ra = rapool.tile([P, 2048], BF16, tag="ra")
            for mc in range(MC):
                pt = psum.tile([P, CHUNK], F32, tag="ps")
                for j in range(4):
                    m0 = mc * CHUNK + j * MM_FREE
                    s = 32 * j
                    nc.tensor.matmul(
                        pt[:, j * MM_FREE : (j + 1) * MM_FREE],
                        lhs_sb[s : s + K, nt * P : (nt + 1) * P],
                        rhs_sb[s : s + K, m0 : m0 + MM_FREE],
                        start=True,
                        stop=True,
                        tile_position=(s, 0),
                    )
                nc.scalar.copy(ck[:, mc * CHUNK : (mc + 1) * CHUNK], pt[:])
                # row-min stream (dist1) on VectorE, bf16 2x rate, FD=2048.
                # nt 0 starts per-chunk so VectorE ramps one drain earlier.
                if nt == 0 and mc < 2:
                    nc.vector.tensor_tensor(
                        ra[:, mc * 1024 : (mc + 1) * 1024],
                        ck[:, mc * CHUNK : mc * CHUNK + 1024],
                        ck[:, mc * CHUNK + 1024 : (mc + 1) * CHUNK],
                        MIN,
                    )
                elif mc == 1:
                    nc.vector.tensor_tensor(
                        ra[:], ck[:, 0:2048], ck[:, 2048:4096], MIN
                    )
                elif mc > 1:
                    nc.vector.tensor_tensor(
                        ra[:], ra[:], ck[:, mc * CHUNK : (mc + 1) * CHUNK], MIN
                    )
            # finalize this n-tile's row-min -> dist1[:, nt]
            # (tensor_tensor_reduce would fuse all of this but wedges the
            #  device on this toolchain -- avoid.)
            rb = rapool.tile([P, 1024], BF16, tag="rb")
            pm = pmpool.tile([P, 512], BF16, tag="pm")
            nc.vector.tensor_tensor(rb[:], ra[:, 0:1024], ra[:, 1024:2048], MIN)
            nc.vector.tensor_tensor(pm[:], rb[:, 0:512], rb[:, 512:1024], MIN)
            nc.vector.tensor_reduce(dist1[:, nt : nt + 1], pm[:], axis=AX, op=MIN)
            # column-min level 1 (n-tile pairs), FD=4096 ops
            if half == 0:
                prev_ck = ck
            else:
                for g in range(2):
                    sl = slice(g * 4096, (g + 1) * 4096)
                    nc.vector.tensor_tensor(
                        prev_ck[:, sl], prev_ck[:, sl], ck[:, sl], MIN
                    )
                pending.append(("seed" if i == 0 else "acc", prev_ck[:]))
            if half == 0:
                flush_pending()

        # Final pair's column merge interleaved with the dist2 tail:
        # per 1024-col group g: L2 merge -> PE transposes -> V reduce, with
        # the L2 of group g+1 keeping VectorE busy during g's transposes.
        assert len(pending) == 1 and pending[0][0] == "acc"
        last_src = pending.pop()[1]

        def l2(g):
            sl = slice(g * 1024, (g + 1) * 1024)
            nc.vector.tensor_tensor(
                colacc[:, sl], colacc[:, sl], last_src[:, sl], MIN
            )

        NG = M // P // 8  # 8 groups of 8 blocks
        l2(0)
        l2(1)
        for g in range(NG):
            if g + 2 < NG:
                l2(g + 2)
            tp = psum.tile([P, 8 * P], BF16, tag="ps")
            for b in range(8):
                t = g * 8 + b
                nc.tensor.transpose(
                    tp[:, b * P : (b + 1) * P],
                    colacc[:, t * P : (t + 1) * P],
                    ident[:],
                )
            nc.vector.tensor_reduce(
                dist2[:, g * 8 : (g + 1) * 8],
                tp[:].rearrange("p (b x) -> p b x", x=P),
                axis=AX,
                op=MIN,
            )

        nc.sync.dma_start(out1_d[:], dist1[:])
        nc.sync.dma_start(out2_d[:], dist2[:])

    nc.compile()
    return nc


def _get_nc():
    global _cached
    if _cached is None:
        _cached = _build()
    return _cached


def _split3(v):
    """Split f64 vector into three bf16 terms summing to v to ~2^-27 rel."""
    h = v.astype(BF)
    r = v - h.astype(np.float64)
    m = r.astype(BF)
    l = (r - m.astype(np.float64)).astype(BF)
    return h, m, l


def _in_maps(xyz1, xyz2):
    xyz1 = np.ascontiguousarray(np.asarray(xyz1, dtype=np.float32))
    xyz2 = np.ascontiguousarray(np.asarray(xyz2, dtype=np.float32))
    maps = []
    for c in range(NCORES):
        b, h = divmod(c, 2)
        X = xyz1[b, h * HALF : (h + 1) * HALF].astype(np.float64)  # [4096, 3]
        Y = xyz2[b].astype(np.float64)  # [8192, 3]

        xh = X.astype(BF)
        xl = (X - xh.astype(np.float64)).astype(BF)
        yh = Y.astype(BF)
        yl = (Y - yh.astype(np.float64)).astype(BF)
        Xr = xh.astype(np.float64) + xl.astype(np.float64)  # representable x
        Yr = yh.astype(np.float64) + yl.astype(np.float64)
        s1h, s1m, s1l = _split3(np.einsum("nd,nd->n", Xr, Xr))
        s2h, s2m, s2l = _split3(np.einsum("md,md->m", Yr, Yr))

        lhs = np.empty((K, HALF), BF)
        lhs[0:3] = 1.0
        lhs[3] = s1h
        lhs[4] = s1m
        lhs[5] = s1l
        lhs[6:9] = (-2.0 * xh.astype(np.float64)).astype(BF).T  # exact *2
        lhs[9:12] = lhs[6:9]
        lhs[12:15] = (-2.0 * xl.astype(np.float64)).astype(BF).T
        lhs[15:18] = lhs[12:15]

        rhs = np.empty((K, M), BF)
        rhs[0] = s2h
        rhs[1] = s2m
        rhs[2] = s2l
        rhs[3:6] = 1.0
        rhs[6:9] = yh.T
        rhs[9:12] = yl.T
        rhs[12:15] = yh.T
        rhs[15:18] = yl.T

        # replicate at partition strips 0/32/64/96 for PE row-tiling
        lhs4 = np.zeros((P, HALF), BF)
        rhs4 = np.zeros((P, M), BF)
        for j in range(4):
            lhs4[32 * j : 32 * j + K] = lhs
            rhs4[32 * j : 32 * j + K] = rhs
        maps.append({"lhs": lhs4, "rhs": rhs4})
    return maps


def _combine(results):
    # dist1: all 8 cores' values are final; out1[p, t] = dist1[t*128 + p]
    d1 = np.concatenate([results[c]["out1"].T.reshape(-1) for c in range(NCORES)])
    # dist2: min over the two half-cores of each batch
    d2 = np.concatenate(
        [
            np.minimum(results[2 * b]["out2"], results[2 * b + 1]["out2"]).T.reshape(-1)
            for b in range(B)
        ]
    )
    val = WEIGHT * (np.float64(d1.mean()) + np.float64(d2.mean())) / 2.0
    return np.float32(val)


def run(xyz1, xyz2, trace=False, **spmd_kwargs):
    """Run on hardware; returns (output_scalar, BassKernelResults)."""
    nc = _get_nc()
    br = run_bass_kernel_spmd(
        nc, _in_maps(xyz1, xyz2), list(range(NCORES)), trace=trace, **spmd_kwargs
    )
    return _combine(br.results), br


def kernel(xyz1, xyz2):
    out, _ = run(xyz1, xyz2)
    return out


if __name__ == "__main__":
    rng = np.random.default_rng(0)
    a = rng.standard_normal((B, N, D)).astype(np.float32)
    b = rng.standard_normal((B, M, D)).astype(np.float32)
    print(kernel(a, b))


# revision 8
# speedup vs baseline: 1.0079x; 1.0078x over previous
"""Chamfer distance L2 (B=4, N=M=8192, D=3) on 8 TRN2 NeuronCores — v2.

Sharding: core c handles batch b = c//2, xyz1-half h = c%2 (4096 query
points against all 8192 xyz2 points of the same batch).

Device kernel (per core, identical SPMD program):
  d[n,m] = ||x1[n]||^2 + ||x2[m]||^2 - 2<x1[n],x2[m]> via ONE K=18
  augmented bf16 matmul per 512-col tile (split-bf16 hi/lo encoding of
  coords and squared norms -> near-f32 accuracy at bf16 PE rate).
  - PE row-tiling: lhs/rhs replicated at partition strips 0/32/64/96;
    the 4 sub-matmuls of each [128 x 2048] PSUM chunk run CONCURRENTLY
    in four 32-row PE strips (tile_position) -> PE busy 248us -> 88us.
  - ScalarE drains PSUM f32 -> SBUF bf16, one [128 x 8192] ck tile per
    n-tile (~212us; at the 1 elem/cycle ACTIVATE floor).
  - VectorE (the binding engine, ~308us busy at 2x_1P tensor_tensor):
    row-min stream per n-tile at FD=2048 + fold + reduce -> dist1[:,nt];
    column-min as level-1 merges of adjacent n-tile pairs (FD=4096)
    plus a pair-lagged accumulate into colacc (keeps V fed while pair
    merges land).
  - Tail: PE transposes colacc 128x128 blocks to PSUM bf16; VectorE
    strided-reduces the old-partition axis -> dist2 partials, with the
    final pair merge interleaved group-by-group for overlap.
  Measured (8-core SPMD): HW exec ~332us, rel err 2.2e-4.

  Rejected offload paths (all probed on this toolchain/HW): SDMA CCE
  accum supports ADD only (min/max fail walrus birverifier); Pool-engine
  TENSOR_TENSOR fails walrus codegen (no Q7 TT opcode); DVE
  tensor_tensor_reduce wedges the device (NRT_EXEC_UNIT_UNRECOVERABLE);
  DVE pool/max8 run at 1x (slower than the 2x tensor_tensor stream).
Host: means + min-combine of the two per-batch halves (O(N) work only).
"""

import sys

for _p in ("/opt/trn_rl_repo",):
    if _p not in sys.path:
        sys.path.insert(0, _p)

from contextlib import ExitStack

import numpy as np
import ml_dtypes

import concourse.bacc as bacc
import concourse.bass as bass
import concourse.mybir as mybir
import concourse.tile as tile
from concourse import masks
from concourse.bass_utils import run_bass_kernel_spmd

WEIGHT = 0.6
B = 4
N = 8192  # xyz1 points per batch
M = 8192  # xyz2 points per batch
D = 3
NCORES = 8
HALF = N // 2  # xyz1 rows per core = 4096

P = 128  # partitions
NT = HALF // P  # 32 n-tiles per core
CHUNK = 2048  # psum chunk free size (4 banks)
MC = M // CHUNK  # 4 m-chunks
MM_FREE = 512  # one PSUM bank of f32
K = 18  # augmented contraction dim (split-bf16)

F32 = mybir.dt.float32
BF16 = mybir.dt.bfloat16
MIN = mybir.AluOpType.min
AX = mybir.AxisListType.X
BF = ml_dtypes.bfloat16

_cached = None


def _build():
    """Build + compile the single-core Bass program (shared by all 8 cores)."""
    nc = bacc.Bacc(
        "TRN2",
        target_bir_lowering=False,
        debug=False,
        enable_asserts=False,
        num_devices=NCORES,
    )

    lhs_d = nc.dram_tensor("lhs", [P, HALF], BF16, kind="ExternalInput")
    rhs_d = nc.dram_tensor("rhs", [P, M], BF16, kind="ExternalInput")
    out1_d = nc.dram_tensor("out1", [P, NT], F32, kind="ExternalOutput")
    out2_d = nc.dram_tensor("out2", [P, M // P], F32, kind="ExternalOutput")

    with tile.TileContext(nc) as tc, ExitStack() as ctx:
        const = ctx.enter_context(tc.tile_pool(name="const", bufs=1))
        ckpool = ctx.enter_context(tc.tile_pool(name="ck", bufs=5))
        rapool = ctx.enter_context(tc.tile_pool(name="ra", bufs=3))
        psum = ctx.enter_context(tc.tile_pool(name="ps", bufs=2, space="PSUM"))

        lhs_sb = const.tile([P, HALF], BF16)
        rhs_sb = const.tile([P, M], BF16)
        ident = const.tile([P, P], BF16)
        colacc = const.tile([P, M], BF16)
        pmstore = const.tile([P, NT * 512], BF16)  # per-n-tile 512-wide folds
        dist1 = const.tile([P, NT], F32)
        dist2 = const.tile([P, M // P], F32)

        # first n-tile's weights + first rhs chunk land first -> early start
        nc.sync.dma_start(lhs_sb[:, 0:P], lhs_d[:, 0:P])
        nc.sync.dma_start(rhs_sb[:, 0:CHUNK], rhs_d[:, 0:CHUNK])
        nc.sync.dma_start(lhs_sb[:, P:HALF], lhs_d[:, P:HALF])
        for mc in range(1, MC):
            nc.sync.dma_start(
                rhs_sb[:, mc * CHUNK : (mc + 1) * CHUNK],
                rhs_d[:, mc * CHUNK : (mc + 1) * CHUNK],
            )
        masks.make_identity(nc, ident[:])

        # pending column ops, emitted with one-pair lag so VectorE keeps
        # row-min work in front of it while L1 pair merges land.
        pending = []

        def flush_pending():
            for kind, src_ap in pending:
                if kind == "seed":
                    nc.vector.tensor_copy(colacc[:], src_ap)
                else:
                    nc.vector.tensor_tensor(colacc[:], colacc[:], src_ap, MIN)
            pending.clear()

        prev_ck = None  # ck handle of the even n-tile of the current pair
        for nt in range(NT):
            i, half = divmod(nt, 2)
            ck = ckpool.tile([P, M], BF16, tag="ck")
            ra = rapool.tile([P, 2048], BF16, tag="ra")
            for mc in range(MC):
                pt = psum.tile([P, CHUNK], F32, tag="ps")
                for j in range(4):
                    m0 = mc * CHUNK + j * MM_FREE
                    s = 32 * j
                    nc.tensor.matmul(
                        pt[:, j * MM_FREE : (j + 1) * MM_FREE],
                        lhs_sb[s : s + K, nt * P : (nt + 1) * P],
                        rhs_sb[s : s + K, m0 : m0 + MM_FREE],
                        start=True,
                        stop=True,
                        tile_position=(s, 0),
                    )
                nc.scalar.copy(ck[:, mc * CHUNK : (mc + 1) * CHUNK], pt[:])
                # last pair: per-chunk column merge so colacc is final the
                # moment the loop ends (frees the tail for transposes only)
                if nt == NT - 1:
                    sl = slice(mc * CHUNK, (mc + 1) * CHUNK)
                    nc.vector.tensor_tensor(
                        prev_ck[:, sl], prev_ck[:, sl], ck[:, sl], MIN
                    )
                    nc.vector.tensor_tensor(
                        colacc[:, sl], colacc[:, sl], prev_ck[:, sl], MIN
                    )
                # row-min stream (dist1) on VectorE, bf16 2x rate, FD=2048.
                # nt 0 starts per-chunk so VectorE ramps one drain earlier.
                if nt == 0 and mc < 2:
                    nc.vector.tensor_tensor(
                        ra[:, mc * 1024 : (mc + 1) * 1024],
                        ck[:, mc * CHUNK : mc * CHUNK + 1024],
                        ck[:, mc * CHUNK + 1024 : (mc + 1) * CHUNK],
                        MIN,
                    )
                elif mc == 1:
                    nc.vector.tensor_tensor(
                        ra[:], ck[:, 0:2048], ck[:, 2048:4096], MIN
                    )
                elif mc > 1:
                    nc.vector.tensor_tensor(
                        ra[:], ra[:], ck[:, mc * CHUNK : (mc + 1) * CHUNK], MIN
                    )
            # fold this n-tile's row-min to 512 wide; the 512->1 reduces are
            # batched into the tail as PE-independent VectorE filler.
            # (tensor_tensor_reduce would fuse all of this but wedges the
            #  device on this toolchain -- avoid.)
            rb = rapool.tile([P, 1024], BF16, tag="rb")
            nc.vector.tensor_tensor(rb[:], ra[:, 0:1024], ra[:, 1024:2048], MIN)
            nc.vector.tensor_tensor(
                pmstore[:, nt * 512 : (nt + 1) * 512],
                rb[:, 0:512],
                rb[:, 512:1024],
                MIN,
            )
            # column-min level 1 (n-tile pairs), one FD=8192 op
            if half == 0:
                prev_ck = ck
            elif nt < NT - 1:
                nc.vector.tensor_tensor(prev_ck[:], prev_ck[:], ck[:], MIN)
                pending.append(("seed" if i == 0 else "acc", prev_ck[:]))
            if half == 0:
                flush_pending()

        # dist2 tail: PE transposes colacc 128x128 blocks, V reduces the
        # old-partition axis; dist1 batch reduces interleave as V filler
        # so V never stalls on the transposes.
        NG = M // P // 8  # 8 groups of 8 blocks
        for g in range(NG):
            tp = psum.tile([P, 8 * P], BF16, tag="ps")
            for b in range(8):
                t = g * 8 + b
                nc.tensor.transpose(
                    tp[:, b * P : (b + 1) * P],
                    colacc[:, t * P : (t + 1) * P],
                    ident[:],
                )
            nc.vector.tensor_reduce(
                dist2[:, g * 8 : (g + 1) * 8],
                tp[:].rearrange("p (b x) -> p b x", x=P),
                axis=AX,
                op=MIN,
            )
            if g < 4:
                nc.vector.tensor_reduce(
                    dist1[:, g * 8 : (g + 1) * 8],
                    pmstore[:, g * 4096 : (g + 1) * 4096].rearrange(
                        "p (t x) -> p t x", x=512
                    ),
                    axis=AX,
                    op=MIN,
                )

        nc.sync.dma_start(out1_d[:], dist1[:])
        nc.sync.dma_start(out2_d[:], dist2[:])

    nc.compile()
    return nc


def _get_nc():
    global _cached
    if _cached is None:
        _cached = _build()
    return _cached


def _split3(v):
    """Split f64 vector into three bf16 terms summing to v to ~2^-27 rel."""
    h = v.astype(BF)
    r = v - h.astype(np.float64)
    m = r.astype(BF)
    l = (r - m.astype(np.float64)).astype(BF)
    return h, m, l


def _in_maps(xyz1, xyz2):
    xyz1 = np.ascontiguousarray(np.asarray(xyz1, dtype=np.float32))
    xyz2 = np.ascontiguousarray(np.asarray(xyz2, dtype=np.float32))
    maps = []
    for c in range(NCORES):
        b, h = divmod(c, 2)
        X = xyz1[b, h * HALF : (h + 1) * HALF].astype(np.float64)  # [4096, 3]
        Y = xyz2[b].astype(np.float64)  # [8192, 3]

        xh = X.astype(BF)
        xl = (X - xh.astype(np.float64)).astype(BF)
        yh = Y.astype(BF)
        yl = (Y - yh.astype(np.float64)).astype(BF)
        Xr = xh.astype(np.float64) + xl.astype(np.float64)  # representable x
        Yr = yh.astype(np.float64) + yl.astype(np.float64)
        s1h, s1m, s1l = _split3(np.einsum("nd,nd->n", Xr, Xr))
        s2h, s2m, s2l = _split3(np.einsum("md,md->m", Yr, Yr))

        lhs = np.empty((K, HALF), BF)
        lhs[0:3] = 1.0
        lhs[3] = s1h
        lhs[4] = s1m
        lhs[5] = s1l
        lhs[6:9] = (-2.0 * xh.astype(np.float64)).astype(BF).T  # exact *2
        lhs[9:12] = lhs[6:9]
        lhs[12:15] = (-2.0 * xl.astype(np.float64)).astype(BF).T
        lhs[15:18] = lhs[12:15]

        rhs = np.empty((K, M), BF)
        rhs[0] = s2h
        rhs[1] = s2m
        rhs[2] = s2l
        rhs[3:6] = 1.0
        rhs[6:9] = yh.T
        rhs[9:12] = yl.T
        rhs[12:15] = yh.T
        rhs[15:18] = yl.T

        # replicate at partition strips 0/32/64/96 for PE row-tiling
        lhs4 = np.zeros((P, HALF), BF)
        rhs4 = np.zeros((P, M), BF)
        for j in range(4):
            lhs4[32 * j : 32 * j + K] = lhs
            rhs4[32 * j : 32 * j + K] = rhs
        maps.append({"lhs": lhs4, "rhs": rhs4})
    return maps


def _combine(results):
    # dist1: all 8 cores' values are final; out1[p, t] = dist1[t*128 + p]
    d1 = np.concatenate([results[c]["out1"].T.reshape(-1) for c in range(NCORES)])
    # dist2: min over the two half-cores of each batch
    d2 = np.concatenate(
        [
            np.minimum(results[2 * b]["out2"], results[2 * b + 1]["out2"]).T.reshape(-1)
            for b in range(B)
        ]
    )
    val = WEIGHT * (np.float64(d1.mean()) + np.float64(d2.mean())) / 2.0
    return np.float32(val)


def run(xyz1, xyz2, trace=False, **spmd_kwargs):
    """Run on hardware; returns (output_scalar, BassKernelResults)."""
    nc = _get_nc()
    br = run_bass_kernel_spmd(
        nc, _in_maps(xyz1, xyz2), list(range(NCORES)), trace=trace, **spmd_kwargs
    )
    return _combine(br.results), br


def kernel(xyz1, xyz2):
    out, _ = run(xyz1, xyz2)
    return out


if __name__ == "__main__":
    rng = np.random.default_rng(0)
    a = rng.standard_normal((B, N, D)).astype(np.float32)
    b = rng.standard_normal((B, M, D)).astype(np.float32)
    print(kernel(a, b))


# revision 26
# speedup vs baseline: 1.0309x; 1.0229x over previous
"""Chamfer distance L2 (B=4, N=M=8192, D=3) on 8 TRN2 NeuronCores — v2.

Sharding: core c handles batch b = c//2, xyz1-half h = c%2 (4096 query
points against all 8192 xyz2 points of the same batch).

Device kernel (per core, identical SPMD program):
  d[n,m] = ||x1[n]||^2 + ||x2[m]||^2 - 2<x1[n],x2[m]> via ONE K=18
  augmented bf16 matmul per 512-col tile (split-bf16 hi/lo encoding of
  coords and squared norms -> near-f32 accuracy at bf16 PE rate).
  - PE row-tiling: lhs/rhs replicated at partition strips 0/32/64/96;
    the 4 sub-matmuls of each [128 x 2048] PSUM chunk run CONCURRENTLY
    in four 32-row PE strips (tile_position) -> PE busy 248us -> 88us.
  - ScalarE drains PSUM f32 -> SBUF bf16, one [128 x 8192] ck tile per
    n-tile (~212us; at the 1 elem/cycle ACTIVATE floor).
  - VectorE (the binding engine, ~308us busy at 2x_1P tensor_tensor):
    row-min stream per n-tile at FD=2048 + fold + reduce -> dist1[:,nt];
    column-min as level-1 merges of adjacent n-tile pairs (FD=4096)
    plus a pair-lagged accumulate into colacc (keeps V fed while pair
    merges land).
  - Tail: PE transposes colacc 128x128 blocks to PSUM bf16; VectorE
    strided-reduces the old-partition axis -> dist2 partials, with the
    final pair merge interleaved group-by-group for overlap.
  Measured (8-core SPMD): HW exec ~322us, rel err 2.2e-4.  (Runs can
  transiently clock-throttle ~19% on all engines -> ~395us; re-measure.)

  Rejected offload paths (all probed on this toolchain/HW): SDMA CCE
  accum supports ADD only (min/max fail walrus birverifier); Pool-engine
  TENSOR_TENSOR fails walrus codegen (no Q7 TT opcode); DVE
  tensor_tensor_reduce wedges the device (NRT_EXEC_UNIT_UNRECOVERABLE);
  DVE pool/max8 run at 1x (slower than the 2x tensor_tensor stream).
Host: means + min-combine of the two per-batch halves (O(N) work only).
"""

import sys

for _p in ("/opt/trn_rl_repo",):
    if _p not in sys.path:
        sys.path.insert(0, _p)

from contextlib import ExitStack

import numpy as np
import ml_dtypes

import concourse.bacc as bacc
import concourse.bass as bass
import concourse.mybir as mybir
import concourse.tile as tile
from concourse import masks
from concourse.bass_utils import run_bass_kernel_spmd

WEIGHT = 0.6
B = 4
N = 8192  # xyz1 points per batch
M = 8192  # xyz2 points per batch
D = 3
NCORES = 8
HALF = N // 2  # xyz1 rows per core = 4096

P = 128  # partitions
NT = HALF // P  # 32 n-tiles per core
CHUNK = 2048  # psum chunk free size (4 banks)
MC = M // CHUNK  # 4 m-chunks
MM_FREE = 512  # one PSUM bank of f32
K = 18  # augmented contraction dim (split-bf16)

F32 = mybir.dt.float32
BF16 = mybir.dt.bfloat16
MIN = mybir.AluOpType.min
AX = mybir.AxisListType.X
BF = ml_dtypes.bfloat16

_cached = None


def _build():
    """Build + compile the single-core Bass program (shared by all 8 cores)."""
    nc = bacc.Bacc(
        "TRN2",
        target_bir_lowering=False,
        debug=False,
        enable_asserts=False,
        num_devices=NCORES,
    )

    lhs_d = nc.dram_tensor("lhs", [P, HALF], BF16, kind="ExternalInput")
    rhs_d = nc.dram_tensor("rhs", [P, M], BF16, kind="ExternalInput")
    out1_d = nc.dram_tensor("out1", [P, NT], F32, kind="ExternalOutput")
    out2_d = nc.dram_tensor("out2", [P, M // P], F32, kind="ExternalOutput")

    with tile.TileContext(nc) as tc, ExitStack() as ctx:
        const = ctx.enter_context(tc.tile_pool(name="const", bufs=1))
        ckpool = ctx.enter_context(tc.tile_pool(name="ck", bufs=5))
        rapool = ctx.enter_context(tc.tile_pool(name="ra", bufs=3))
        r4pool = ctx.enter_context(tc.tile_pool(name="ra4", bufs=2))
        scpool = ctx.enter_context(tc.tile_pool(name="sc", bufs=2))
        psum = ctx.enter_context(tc.tile_pool(name="ps", bufs=2, space="PSUM"))

        lhs_sb = const.tile([P, HALF], BF16)
        rhs_sb = const.tile([P, M], BF16)
        ident = const.tile([P, P], BF16)
        colacc = const.tile([P, M], BF16)
        pmstore = const.tile([P, NT * 512], BF16)  # per-n-tile 512-wide folds
        dist1 = const.tile([P, NT], F32)
        dist2 = const.tile([P, M // P], F32)

        # first n-tile's weights + first rhs chunk land first -> early start
        nc.sync.dma_start(lhs_sb[:, 0:P], lhs_d[:, 0:P])
        nc.sync.dma_start(rhs_sb[:, 0:CHUNK], rhs_d[:, 0:CHUNK])
        nc.sync.dma_start(lhs_sb[:, P:HALF], lhs_d[:, P:HALF])
        for mc in range(1, MC):
            nc.sync.dma_start(
                rhs_sb[:, mc * CHUNK : (mc + 1) * CHUNK],
                rhs_d[:, mc * CHUNK : (mc + 1) * CHUNK],
            )
        masks.make_identity(nc, ident[:])

        def dist1_fold(h):
            """dist1[:, 8h:8h+8] = per-segment min of pmstore group h, via
            2x-rate strided TT folds (tensor_reduce is 1x -- ~25% slower)."""
            sc = scpool.tile([P, 4096], BF16, tag="sc")
            v = pmstore[:, h * 4096 : (h + 1) * 4096].rearrange(
                "p (t x) -> p t x", x=512
            )
            cur, other = sc[:, 0:2048], sc[:, 2048:4096]
            nc.vector.tensor_tensor(
                cur.rearrange("p (t x) -> p t x", x=256),
                v[:, :, 0:256],
                v[:, :, 256:512],
                MIN,
            )
            w = 256
            while w > 8:
                half = w // 2
                cv = cur[:, 0 : 8 * w].rearrange("p (t x) -> p t x", x=w)
                ov = other[:, 0 : 8 * half].rearrange("p (t x) -> p t x", x=half)
                nc.vector.tensor_tensor(ov, cv[:, :, 0:half], cv[:, :, half:w], MIN)
                cur, other = other, cur
                w = half
            nc.vector.tensor_reduce(
                dist1[:, h * 8 : (h + 1) * 8],
                cur[:, 0:64].rearrange("p (t x) -> p t x", x=8),
                axis=AX,
                op=MIN,
            )

        prev_ck = None  # ck handle of the even n-tile of the current pair
        for nt in range(NT):
            i, half = divmod(nt, 2)
            # n-tile 0 is drained straight into colacc (seeds the column
            # min for free); later n-tiles go to the ck ring.
            ck = colacc if nt == 0 else ckpool.tile([P, M], BF16, tag="ck")
            ra = rapool.tile([P, 2048], BF16, tag="ra")
            for mc in range(MC):
                pt = psum.tile([P, CHUNK], F32, tag="ps")
                for j in range(4):
                    m0 = mc * CHUNK + j * MM_FREE
                    s = 32 * j
                    nc.tensor.matmul(
                        pt[:, j * MM_FREE : (j + 1) * MM_FREE],
                        lhs_sb[s : s + K, nt * P : (nt + 1) * P],
                        rhs_sb[s : s + K, m0 : m0 + MM_FREE],
                        start=True,
                        stop=True,
                        tile_position=(s, 0),
                    )
                nc.scalar.copy(ck[:, mc * CHUNK : (mc + 1) * CHUNK], pt[:])
                # last pair: per-chunk column merge so colacc is final the
                # moment the loop ends (frees the tail for transposes only)
                if nt == NT - 1:
                    sl = slice(mc * CHUNK, (mc + 1) * CHUNK)
                    nc.vector.tensor_tensor(
                        prev_ck[:, sl], prev_ck[:, sl], ck[:, sl], MIN
                    )
                    nc.vector.tensor_tensor(
                        colacc[:, sl], colacc[:, sl], prev_ck[:, sl], MIN
                    )
                # early n-tiles: per-chunk row-min so VectorE ramps while
                # ScalarE's run-ahead is still building.
                if nt < 4 and mc < 2:
                    nc.vector.tensor_tensor(
                        ra[:, mc * 1024 : (mc + 1) * 1024],
                        ck[:, mc * CHUNK : mc * CHUNK + 1024],
                        ck[:, mc * CHUNK + 1024 : (mc + 1) * CHUNK],
                        MIN,
                    )
                elif nt < 4 and mc > 1:
                    nc.vector.tensor_tensor(
                        ra[:], ra[:], ck[:, mc * CHUNK : (mc + 1) * CHUNK], MIN
                    )
            # row-min (dist1) on VectorE, bf16 2x rate: one all-fresh
            # FD=4096 op over the full tile, then fold to 512 wide; the
            # 512->1 reduces are batched into the tail as PE-independent
            # VectorE filler.  (tensor_tensor_reduce would fuse this but
            # wedges the device on this toolchain -- avoid.)
            rb = rapool.tile([P, 1024], BF16, tag="rb")
            if nt >= 4:
                ra4 = r4pool.tile([P, 4096], BF16, tag="ra4")
                nc.vector.tensor_tensor(
                    ra4[:], ck[:, 0:4096], ck[:, 4096:8192], MIN
                )
                nc.vector.tensor_tensor(
                    ra[:], ra4[:, 0:2048], ra4[:, 2048:4096], MIN
                )
            nc.vector.tensor_tensor(rb[:], ra[:, 0:1024], ra[:, 1024:2048], MIN)
            nc.vector.tensor_tensor(
                pmstore[:, nt * 512 : (nt + 1) * 512],
                rb[:, 0:512],
                rb[:, 512:1024],
                MIN,
            )
            # column-min (n-tile pairs): level-1 merge + immediate
            # accumulate into colacc, one FD=8192 op each.  Pair 0's ck is
            # colacc itself, so its merge is a single op with no seed copy.
            if half == 0 and nt > 0:
                prev_ck = ck
            elif nt == 1:
                nc.vector.tensor_tensor(colacc[:], colacc[:], ck[:], MIN)
            elif half == 1 and nt < NT - 1:
                nc.vector.tensor_tensor(prev_ck[:], prev_ck[:], ck[:], MIN)
                nc.vector.tensor_tensor(colacc[:], colacc[:], prev_ck[:], MIN)
            # dist1 batch reduce for n-tile group h drains mid-loop, as
            # soon as its 8 pmstore segments are complete (keeps the tail
            # short; group 3 finishes at nt 31 and stays in the tail).
            if nt in (9, 17, 25):
                dist1_fold((nt - 2) // 8)

        # dist2 tail: PE transposes colacc 128x128 blocks, V reduces the
        # old-partition axis; dist1 batch reduces interleave as V filler
        # so V never stalls on the transposes.
        NG = M // P // 8  # 8 groups of 8 blocks
        for g in range(NG):
            tp = psum.tile([P, 8 * P], BF16, tag="ps")
            for b in range(8):
                t = g * 8 + b
                nc.tensor.transpose(
                    tp[:, b * P : (b + 1) * P],
                    colacc[:, t * P : (t + 1) * P],
                    ident[:],
                )
            nc.vector.tensor_reduce(
                dist2[:, g * 8 : (g + 1) * 8],
                tp[:].rearrange("p (b x) -> p b x", x=P),
                axis=AX,
                op=MIN,
            )
            if g == 1:
                dist1_fold(3)

        nc.sync.dma_start(out1_d[:], dist1[:])
        nc.sync.dma_start(out2_d[:], dist2[:])

    nc.compile()
    return nc


def _get_nc():
    global _cached
    if _cached is None:
        _cached = _build()
    return _cached


def _split3(v):
    """Split f64 vector into three bf16 terms summing to v to ~2^-27 rel."""
    h = v.astype(BF)
    r = v - h.astype(np.float64)
    m = r.astype(BF)
    l = (r - m.astype(np.float64)).astype(BF)
    return h, m, l


def _in_maps(xyz1, xyz2):
    xyz1 = np.ascontiguousarray(np.asarray(xyz1, dtype=np.float32))
    xyz2 = np.ascontiguousarray(np.asarray(xyz2, dtype=np.float32))
    maps = []
    for c in range(NCORES):
        b, h = divmod(c, 2)
        X = xyz1[b, h * HALF : (h + 1) * HALF].astype(np.float64)  # [4096, 3]
        Y = xyz2[b].astype(np.float64)  # [8192, 3]

        xh = X.astype(BF)
        xl = (X - xh.astype(np.float64)).astype(BF)
        yh = Y.astype(BF)
        yl = (Y - yh.astype(np.float64)).astype(BF)
        Xr = xh.astype(np.float64) + xl.astype(np.float64)  # representable x
        Yr = yh.astype(np.float64) + yl.astype(np.float64)
        s1h, s1m, s1l = _split3(np.einsum("nd,nd->n", Xr, Xr))
        s2h, s2m, s2l = _split3(np.einsum("md,md->m", Yr, Yr))

        lhs = np.empty((K, HALF), BF)
        lhs[0:3] = 1.0
        lhs[3] = s1h
        lhs[4] = s1m
        lhs[5] = s1l
        lhs[6:9] = (-2.0 * xh.astype(np.float64)).astype(BF).T  # exact *2
        lhs[9:12] = lhs[6:9]
        lhs[12:15] = (-2.0 * xl.astype(np.float64)).astype(BF).T
        lhs[15:18] = lhs[12:15]

        rhs = np.empty((K, M), BF)
        rhs[0] = s2h
        rhs[1] = s2m
        rhs[2] = s2l
        rhs[3:6] = 1.0
        rhs[6:9] = yh.T
        rhs[9:12] = yl.T
        rhs[12:15] = yh.T
        rhs[15:18] = yl.T

        # replicate at partition strips 0/32/64/96 for PE row-tiling
        lhs4 = np.zeros((P, HALF), BF)
        rhs4 = np.zeros((P, M), BF)
        for j in range(4):
            lhs4[32 * j : 32 * j + K] = lhs
            rhs4[32 * j : 32 * j + K] = rhs
        maps.append({"lhs": lhs4, "rhs": rhs4})
    return maps


def _combine(results):
    # dist1: all 8 cores' values are final; out1[p, t] = dist1[t*128 + p]
    d1 = np.concatenate([results[c]["out1"].T.reshape(-1) for c in range(NCORES)])
    # dist2: min over the two half-cores of each batch
    d2 = np.concatenate(
        [
            np.minimum(results[2 * b]["out2"], results[2 * b + 1]["out2"]).T.reshape(-1)
            for b in range(B)
        ]
    )
    val = WEIGHT * (np.float64(d1.mean()) + np.float64(d2.mean())) / 2.0
    return np.float32(val)


def run(xyz1, xyz2, trace=False, **spmd_kwargs):
    """Run on hardware; returns (output_scalar, BassKernelResults)."""
    nc = _get_nc()
    br = run_bass_kernel_spmd(
        nc, _in_maps(xyz1, xyz2), list(range(NCORES)), trace=trace, **spmd_kwargs
    )
    return _combine(br.results), br


def kernel(xyz1, xyz2):
    out, _ = run(xyz1, xyz2)
    return out


if __name__ == "__main__":
    rng = np.random.default_rng(0)
    a = rng.standard_normal((B, N, D)).astype(np.float32)
    b = rng.standard_normal((B, M, D)).astype(np.float32)
    print(kernel(a, b))
